# revision 4
# baseline (speedup 1.0000x reference)
"""CRF input-energy kernel for Trainium2 (8 NeuronCores, data-parallel on batch).

Computes out[B,T,U] = X @ kernel + bias, with left/right boundary energies
added at t=0 and t=T-1.

Strategy: pure data parallel — each of the 8 cores gets 8 of the 64 batch
sequences. Host-side we relayout each core's X shard to d-major [D, R]
(R = 8*4096 rows) so the contraction dim D=128 lands on SBUF partitions with
fully contiguous DMA. Per 128-row block the TensorEngine computes
psum = Xt_block.T @ W (natural [rows, U] layout), the VectorEngine adds the
(pre-broadcast) bias while copying PSUM->SBUF, and boundary rows get two tiny
single-partition adds. The blocked output [128, NBLK*U] is un-permuted on host.
"""

import numpy as np

import concourse.bass as bass
import concourse.tile as tile
from concourse import bacc, mybir
from concourse.bass import ds, ts
from concourse.bass_utils import run_bass_kernel_spmd

B, T, D, U = 64, 4096, 128, 32
N_CORES = 8
SEQ_PER_CORE = B // N_CORES      # 8
R = SEQ_PER_CORE * T             # 32768 rows per core
PB = 128                         # rows per block (partition dim)
NBLK = R // PB                   # 256 blocks per core
CH = 4096                        # xt columns (rows of X) per DMA chunk
NCH = R // CH                    # chunks per core
BLK_PER_CH = CH // PB            # 32 blocks per chunk
PSUM_N = 512                     # psum tile free size (one bank)
BLK_PER_PS = PSUM_N // U         # 16 blocks per psum tile
S_PER_CH = BLK_PER_CH // BLK_PER_PS
TBLK = T // PB                   # blocks per sequence (32)

_NC_CACHE = {}


def _build():
    nc = bacc.Bacc(
        "TRN2", target_bir_lowering=False, debug=False, num_devices=N_CORES
    )
    f32 = mybir.dt.float32
    xt = nc.dram_tensor("xt", [PB, R], f32, kind="ExternalInput").ap()
    w = nc.dram_tensor("w", [D, U], f32, kind="ExternalInput").ap()
    btabl = nc.dram_tensor("btabl", [PB, PSUM_N], f32, kind="ExternalInput").ap()
    btabr = nc.dram_tensor("btabr", [PB, PSUM_N], f32, kind="ExternalInput").ap()
    out = nc.dram_tensor("out", [PB, NBLK * U], f32, kind="ExternalOutput").ap()

    with tile.TileContext(nc) as tc:
        with (
            tc.tile_pool(name="consts", bufs=1) as consts,
            tc.tile_pool(name="xin", bufs=3) as xin,
            tc.tile_pool(name="outp", bufs=3) as outp,
            tc.tile_pool(name="ps", bufs=6, space=bass.MemorySpace.PSUM) as psp,
        ):
            w_sb = consts.tile([D, U], f32)
            nc.sync.dma_start(w_sb[:], w[:])
            btabl_sb = consts.tile([PB, PSUM_N], f32)
            nc.sync.dma_start(btabl_sb[:], btabl[:])
            btabr_sb = consts.tile([PB, PSUM_N], f32)
            nc.sync.dma_start(btabr_sb[:], btabr[:])

            for n in range(NCH):
                xt_t = xin.tile([PB, CH], f32)
                nc.sync.dma_start(xt_t[:], xt[:, ds(n * CH, CH)])
                o_t = outp.tile([PB, BLK_PER_CH * U], f32)
                for s in range(S_PER_CH):
                    ps = psp.tile([PB, PSUM_N], f32)
                    for j in range(BLK_PER_PS):
                        bl = s * BLK_PER_PS + j
                        nc.tensor.matmul(
                            ps[:, ts(j, U)],
                            xt_t[:, ts(bl, PB)],
                            w_sb[:],
                            start=True,
                            stop=True,
                        )
                    # Each psum tile spans 2048 rows; a 4096-row sequence is
                    # exactly two tiles, so even tiles hold the t=0 row at
                    # (p=0, col 0:U) and odd tiles the t=T-1 row at
                    # (p=127, col PSUM_N-U:PSUM_N). The bias tables carry the
                    # boundary energies at those spots.
                    tab = btabl_sb if s % 2 == 0 else btabr_sb
                    nc.vector.tensor_add(o_t[:, ts(s, PSUM_N)], ps[:], tab[:])
                nc.scalar.dma_start(
                    out[:, ds(n * BLK_PER_CH * U, BLK_PER_CH * U)], o_t[:]
                )
    nc.compile()
    return nc


def _get_nc():
    if "nc" not in _NC_CACHE:
        _NC_CACHE["nc"] = _build()
    return _NC_CACHE["nc"]


def _make_in_maps(X, kern, bias, left_boundary, right_boundary):
    X = np.ascontiguousarray(np.asarray(X, dtype=np.float32))
    w = np.ascontiguousarray(np.asarray(kern, dtype=np.float32))
    bias = np.asarray(bias, dtype=np.float32)
    lb = np.asarray(left_boundary, dtype=np.float32)
    rb = np.asarray(right_boundary, dtype=np.float32)
    btabl = np.tile(bias[None, :], (PB, BLK_PER_PS))
    btabr = btabl.copy()
    btabl[0, 0:U] += lb
    btabr[PB - 1, PSUM_N - U : PSUM_N] += rb
    btabl = np.ascontiguousarray(btabl)
    btabr = np.ascontiguousarray(btabr)
    in_maps = []
    for c in range(N_CORES):
        Xc = X[c * SEQ_PER_CORE : (c + 1) * SEQ_PER_CORE].reshape(R, D)
        xt = np.ascontiguousarray(Xc.T)
        in_maps.append({"xt": xt, "w": w, "btabl": btabl, "btabr": btabr})
    return in_maps


def _unshard(results):
    outs = []
    for c in range(N_CORES):
        o = np.asarray(results[c]["out"])  # [PB, NBLK*U]
        e = o.reshape(PB, NBLK, U).transpose(1, 0, 2).reshape(SEQ_PER_CORE, T, U)
        outs.append(e)
    return np.concatenate(outs, axis=0)


def _run(inputs, trace=False):
    nc = _get_nc()
    in_maps = _make_in_maps(
        inputs["X"],
        inputs["kernel"],
        inputs["bias"],
        inputs["left_boundary"],
        inputs["right_boundary"],
    )
    res = run_bass_kernel_spmd(nc, in_maps, list(range(N_CORES)), trace=trace)
    return _unshard(res.results), res


def kernel(X, kernel, bias, left_boundary, right_boundary):
    out, _ = _run(
        {
            "X": X,
            "kernel": kernel,
            "bias": bias,
            "left_boundary": left_boundary,
            "right_boundary": right_boundary,
        }
    )
    return out


# revision 5
# speedup vs baseline: 1.9344x; 1.9344x over previous
"""CRF input-energy kernel for Trainium2 (8 NeuronCores, data-parallel on batch).

Computes out[B,T,U] = X @ kernel + bias, with left/right boundary energies
added at t=0 and t=T-1.

Strategy: pure data parallel — each of the 8 cores gets 8 of the 64 batch
sequences. Host-side we relayout each core's X shard to d-major [D, R]
(R = 8*4096 rows) so the contraction dim D=128 lands on SBUF partitions with
fully contiguous DMA. The weight [128,32] stays stationary in the PE array
(replicated into all four 32-column groups via tile_position), and X streams
through as the 512-wide moving operand: each matmul produces a transposed
energy block [32u, 512r] in one of four PSUM partition groups. The
VectorEngine adds bias (pre-broadcast per-partition tables, with the t=0 /
t=T-1 boundary energies folded into the even/odd-tile variants) while copying
PSUM->SBUF. The blocked transposed output [128, R*U/128] is un-permuted on
host.
"""

import numpy as np

import concourse.bass as bass
import concourse.tile as tile
from concourse import bacc, mybir
from concourse.bass import ds, ts
from concourse.bass_utils import run_bass_kernel_spmd

B, T, D, U = 64, 4096, 128, 32
N_CORES = 8
SEQ_PER_CORE = B // N_CORES      # 8
R = SEQ_PER_CORE * T             # 32768 rows per core
PB = 128                         # SBUF partition count
MOV = 512                        # moving-operand width (rows per matmul)
GRP = PB // U                    # 4 PE column groups / PSUM partition groups
ROWS_PER_PS = GRP * MOV          # 2048 rows per psum tile
NPS = R // ROWS_PER_PS           # 16 psum tiles per core
CH = 4096                        # X rows per DMA chunk (= one sequence)
NCH = R // CH                    # 8 chunks per core
PS_PER_CH = CH // ROWS_PER_PS    # 2 psum tiles per chunk
OUT_COLS = R * U // PB           # 8192 output columns on device

_NC_CACHE = {}


def _build():
    nc = bacc.Bacc(
        "TRN2", target_bir_lowering=False, debug=False, num_devices=N_CORES
    )
    f32 = mybir.dt.float32
    xt = nc.dram_tensor("xt", [PB, R], f32, kind="ExternalInput").ap()
    w = nc.dram_tensor("w", [D, U], f32, kind="ExternalInput").ap()
    btabl = nc.dram_tensor("btabl", [PB, MOV], f32, kind="ExternalInput").ap()
    btabr = nc.dram_tensor("btabr", [PB, MOV], f32, kind="ExternalInput").ap()
    out = nc.dram_tensor("out", [PB, OUT_COLS], f32, kind="ExternalOutput").ap()

    with tile.TileContext(nc) as tc:
        with (
            tc.tile_pool(name="consts", bufs=1) as consts,
            tc.tile_pool(name="xin", bufs=3) as xin,
            tc.tile_pool(name="outp", bufs=3) as outp,
            tc.tile_pool(name="ps", bufs=6, space=bass.MemorySpace.PSUM) as psp,
        ):
            w_sb = consts.tile([D, U], f32)
            nc.sync.dma_start(w_sb[:], w[:])
            btabl_sb = consts.tile([PB, MOV], f32)
            nc.sync.dma_start(btabl_sb[:], btabl[:])
            btabr_sb = consts.tile([PB, MOV], f32)
            nc.sync.dma_start(btabr_sb[:], btabr[:])

            for n in range(NCH):
                xt_t = xin.tile([PB, CH], f32)
                nc.sync.dma_start(xt_t[:], xt[:, ds(n * CH, CH)])
                o_t = outp.tile([PB, PS_PER_CH * MOV], f32)
                for s in range(PS_PER_CH):
                    ps = psp.tile([PB, MOV], f32)
                    for g in range(GRP):
                        blk = s * GRP + g  # 512-row block within chunk
                        nc.tensor.matmul(
                            ps[g * U : (g + 1) * U, :],
                            w_sb[:],
                            xt_t[:, ds(blk * MOV, MOV)],
                            start=True,
                            stop=True,
                            tile_position=(0, g * U),
                        )
                    # psum tile = 2048 rows; a 4096-row sequence is exactly two
                    # tiles: even tiles hold the t=0 row at (partitions 0:32,
                    # col 0), odd tiles the t=T-1 row at (partitions 96:128,
                    # col 511). The bias tables carry the boundary energies.
                    tab = btabl_sb if s % 2 == 0 else btabr_sb
                    nc.vector.tensor_add(o_t[:, ts(s, MOV)], ps[:], tab[:])
                nc.scalar.dma_start(
                    out[:, ds(n * PS_PER_CH * MOV, PS_PER_CH * MOV)], o_t[:]
                )
    nc.compile()
    return nc


def _get_nc():
    if "nc" not in _NC_CACHE:
        _NC_CACHE["nc"] = _build()
    return _NC_CACHE["nc"]


def _make_in_maps(X, kern, bias, left_boundary, right_boundary):
    X = np.ascontiguousarray(np.asarray(X, dtype=np.float32))
    w = np.ascontiguousarray(np.asarray(kern, dtype=np.float32))
    bias = np.asarray(bias, dtype=np.float32)
    lb = np.asarray(left_boundary, dtype=np.float32)
    rb = np.asarray(right_boundary, dtype=np.float32)
    base = np.repeat(np.tile(bias, GRP)[:, None], MOV, axis=1)  # [128, 512]
    btabl = base.copy()
    btabl[0:U, 0] += lb
    btabr = base.copy()
    btabr[PB - U : PB, MOV - 1] += rb
    btabl = np.ascontiguousarray(btabl)
    btabr = np.ascontiguousarray(btabr)
    in_maps = []
    for c in range(N_CORES):
        Xc = X[c * SEQ_PER_CORE : (c + 1) * SEQ_PER_CORE].reshape(R, D)
        xt = np.ascontiguousarray(Xc.T)
        in_maps.append({"xt": xt, "w": w, "btabl": btabl, "btabr": btabr})
    return in_maps


def _unshard(results):
    outs = []
    for c in range(N_CORES):
        o = np.asarray(results[c]["out"])  # [128, OUT_COLS]
        # partition p = 32g + u ; column = 512k + c ; row = 2048k + 512g + c
        e = (
            o.reshape(GRP, U, NPS, MOV)
            .transpose(2, 0, 3, 1)
            .reshape(SEQ_PER_CORE, T, U)
        )
        outs.append(e)
    return np.concatenate(outs, axis=0)


def _run(inputs, trace=False):
    nc = _get_nc()
    in_maps = _make_in_maps(
        inputs["X"],
        inputs["kernel"],
        inputs["bias"],
        inputs["left_boundary"],
        inputs["right_boundary"],
    )
    res = run_bass_kernel_spmd(nc, in_maps, list(range(N_CORES)), trace=trace)
    return _unshard(res.results), res


def kernel(X, kernel, bias, left_boundary, right_boundary):
    out, _ = _run(
        {
            "X": X,
            "kernel": kernel,
            "bias": bias,
            "left_boundary": left_boundary,
            "right_boundary": right_boundary,
        }
    )
    return out
